# revision 1
# baseline (speedup 1.0000x reference)
"""Haar-DWT L1 loss (DWTLoss) on 8 trn2 NeuronCores.

Math: the 2D haar DWT is linear, so p_coeffs - t_coeffs = haar(pred - target).
For each 2x2 block of d = pred - target with entries (a b / c d) the four
(unnormalized) subband values are
    s1 = a+b+c+d, s2 = a+b-c-d, s3 = a-b+c-d, s4 = a-b-c+d
and the block's loss contribution is 0.5*(|s1|+|s2|+|s3|+|s4|).
With u_e = a+c, u_o = b+d (vertical sums) and v_e = a-c, v_o = b-d
(vertical diffs): s1 = u_e+u_o, s3 = u_e-u_o, s2 = v_e+v_o, s4 = v_e-v_o,
so the horizontal stage is one add + one subtract over the packed uv tile
and ACT finishes with Abs + accum_out.  (The |x+y|+|x-y| = 2*max(|x|,|y|)
collapse via AluOpType.abs_max does NOT compile on this walrus build.)

Layout: each core's [4,3,512,512] shard is viewed flat as [128, 24576]:
partition p holds 48 *consecutive* image rows (96 KiB contiguous HBM), so
every DMA descriptor is a multi-KiB contiguous run (near-peak HBM rate)
and vertical 2x2 neighbors sit in the SAME partition at free-dim offset W.

Engine split per chunk of CH columns (CH multiple of 1024 = one row pair):
  - DMA:  pt/tt [128, CH] tiles, 8 KiB/partition descriptors at CH=2048.
  - Pool: d = pred - target  (the last chunk's subtract runs on DVE so the
          tail does not serialize behind Pool's drain).
  - DVE:  u = d_top + d_bot, v = d_top - d_bot (free-dim offset views into
          one packed uv tile), then s_add = uv_e + uv_o, s_sub = uv_e - uv_o
          (= s1|s2 and s3|s4 stacked).
  - ACT:  activation(Abs) with accum_out -> per-partition partial sums.
  - PE:   final 128-way partition sum of the partials via a ones-vector
          matmul, so the output DMA is a single-descriptor [1, n] store
          (a [128, n] store costs ~95ns/partition of desc-gen on SP).
The pipeline is DMA-bound (~70 us floor at ~360 GB/s); Pool ~49 us,
DVE ~51 us, ACT ~21 us.
Host: sum the per-core [1, n] partials, divide by 2*N_SUB.

This walrus build allows only ONE embedded sync-wait per instruction, so
`_hoist_excess_waits` moves extra waits onto standalone EventSemaphore
instructions on the same engine stream (semantics preserved: the engine
executes them in order before the instruction).

Sharding: pure data parallel over the batch dim (4 images per core); the
host reduces the 8 tiny partial tiles (the "all-reduce" of the hint).
"""

import os

import numpy as np

import concourse.bass as bass
import concourse.mybir as mybir
from concourse.bass_utils import run_bass_kernel_spmd
from concourse.tile import TileContext

B, C, H, W = 32, 3, 512, 512
N_CORES = 8
B_LOC = B // N_CORES                        # batch shard per core
N_SUB = B * C * (H // 2) * (W // 2)         # elements per DWT subband
P = 128                                     # SBUF partitions
NFREE = (B_LOC * C * H * W) // P            # 24576 f32 per partition

F32 = mybir.dt.float32
ALU = mybir.AluOpType

# chunk sizes in f32 columns; each must be a multiple of 1024 (= one full
# row pair) and they must sum to NFREE.  Small chunks at the start let
# compute begin early; small chunks at the end shorten the post-DMA tail.
SIZES = [1024, 1024] + [2048] * 10 + [1024, 1024]


def _hoist_excess_waits(nc):
    """Walrus in this toolchain allows one embedded sync-wait per
    instruction.  Tile sometimes attaches 2-3 (cross-engine + self + DMA).
    Hoist all but the last wait onto standalone same-engine EventSemaphore
    instructions inserted immediately before the offender — the engine
    stream executes them in order, so the AND-of-waits semantics and every
    sem value are preserved.  HW-compile path only: the injected bare
    instructions lack CoreSim bookkeeping (use _build() output for sim)."""
    n = 0
    for f in nc.m.functions:
        for bb in f.blocks:
            out = []
            for ins in bb.instructions:
                si = getattr(ins, "sync_info", None)
                ow = list(si.on_wait) if (si is not None and si.on_wait) else []
                if len(ow) > 1 and not isinstance(ins, mybir.InstEventSemaphore):
                    for w in ow[:-1]:
                        ev = mybir.InstEventSemaphore(name=f"{ins.name}-hw{n}")
                        n += 1
                        ev.engine = ins.engine
                        ev.sync_info = mybir.SyncInfo(on_wait=[w], on_update=[])
                        out.append(ev)
                    ins.sync_info = mybir.SyncInfo(
                        on_wait=[ow[-1]], on_update=list(si.on_update or [])
                    )
                out.append(ins)
            if n:
                bb.instructions[:] = out
    return nc


def _build(sizes=None, io_bufs=3, wk_bufs=2, loop_n=None):
    nc = bass.Bass()
    pred = nc.dram_tensor("pred", [B_LOC, C, H, W], F32, kind="ExternalInput")
    targ = nc.dram_tensor("target", [B_LOC, C, H, W], F32, kind="ExternalInput")

    # partition p = 48 consecutive image rows (96 KiB contiguous in HBM)
    pf = pred[:].flatten().rearrange("(p n) -> p n", p=P, n=NFREE)
    tf = targ[:].flatten().rearrange("(p n) -> p n", p=P, n=NFREE)

    if sizes is None:
        sizes = SIZES
    assert sum(sizes) == NFREE and all(s % (2 * W) == 0 for s in sizes), sizes
    n_chunks = len(sizes)
    chmax = max(sizes)

    # partial sums leave the chip as single-partition rows: a [128, n] tile
    # DMA costs ~95ns of desc-gen per partition on the SP sequencer (~12 us),
    # so the 128-way partition sum is done on the (otherwise idle) PE with a
    # ones-vector matmul and only [1, n] is stored (one descriptor).
    out_a = nc.dram_tensor("partial_a", [1, n_chunks - 1], F32,
                           kind="ExternalOutput")
    out_b = nc.dram_tensor("partial_b", [1, 1], F32, kind="ExternalOutput")

    with TileContext(nc) as tc:
        with (
            tc.tile_pool(name="io", bufs=io_bufs) as io,
            tc.tile_pool(name="dp", bufs=io_bufs) as dp,
            tc.tile_pool(name="wk", bufs=wk_bufs) as wk,
            tc.tile_pool(name="ps", bufs=1, space="PSUM") as ps,
            tc.tile_pool(name="cst", bufs=1) as cst,
        ):
            acc_a = cst.tile([P, n_chunks - 1], F32)
            acc_b = cst.tile([P, 1], F32)
            ones = cst.tile([P, 1], F32)
            nc.vector.memset(ones[:], 1.0)
            red = ps.tile([P, n_chunks], F32)
            red_sb = cst.tile([P, n_chunks], F32)

            def body():
                c0 = 0
                for bi, ch in enumerate(sizes):
                    hf = ch // 2
                    last = bi == n_chunks - 1
                    pt = io.tile([P, chmax], F32, tag="pt")
                    tt = io.tile([P, chmax], F32, tag="tt")
                    nc.sync.dma_start(pt[:, :ch], pf[:, c0 : c0 + ch])
                    nc.sync.dma_start(tt[:, :ch], tf[:, c0 : c0 + ch])

                    # overlap the bulk of the partials reduce+store with the
                    # last chunk's compute; only acc_b stays on the tail
                    if last:
                        nc.tensor.matmul(
                            red[0:1, : n_chunks - 1],
                            ones[:],
                            acc_a[:],
                            start=True,
                            stop=True,
                        )
                        nc.scalar.copy(
                            red_sb[0:1, : n_chunks - 1],
                            red[0:1, : n_chunks - 1],
                        )
                        nc.sync.dma_start(
                            out_a[:], red_sb[0:1, : n_chunks - 1]
                        )

                    # the final chunk's subtract runs on DVE: at the tail the
                    # Pool->DVE handoff would serialize behind Pool's drain
                    d = dp.tile([P, chmax], F32, tag="d")
                    sub_eng = nc.vector if last else nc.gpsimd
                    sub_eng.tensor_tensor(
                        d[:, :ch], pt[:, :ch], tt[:, :ch], ALU.subtract
                    )

                    # vertical pass: partition-local row pairs at offset W
                    uv = wk.tile([P, chmax], F32, tag="uv")
                    dv = d[:, :ch].rearrange(
                        "p (k two w) -> p k two w", two=2, w=W
                    )
                    uvw = uv[:, :ch].rearrange(
                        "p (half k w) -> p half k w", half=2, w=W
                    )
                    nc.vector.tensor_tensor(
                        uvw[:, 0], dv[:, :, 0, :], dv[:, :, 1, :], ALU.add
                    )
                    nc.vector.tensor_tensor(
                        uvw[:, 1], dv[:, :, 0, :], dv[:, :, 1, :], ALU.subtract
                    )

                    # horizontal: s_add = uv_e + uv_o (= s1 | s2 stacked),
                    #             s_sub = uv_e - uv_o (= s3 | s4 stacked)
                    s2 = wk.tile([P, chmax], F32, tag="s2")
                    uvv = uv[:, :ch].rearrange("p (n two) -> p n two", two=2)
                    nc.vector.tensor_tensor(
                        s2[:, :hf], uvv[:, :, 0], uvv[:, :, 1], ALU.add
                    )
                    nc.vector.tensor_tensor(
                        s2[:, hf:ch], uvv[:, :, 0], uvv[:, :, 1], ALU.subtract
                    )

                    # abs + per-partition sum on ACT
                    scr = wk.tile([P, chmax], F32, tag="scr")
                    acol = acc_b[:, 0:1] if last else acc_a[:, bi : bi + 1]
                    nc.scalar.activation(
                        scr[:, :ch],
                        s2[:, :ch],
                        mybir.ActivationFunctionType.Abs,
                        accum_out=acol,
                    )
                    c0 += ch
                nc.tensor.matmul(
                    red[0:1, n_chunks - 1 : n_chunks],
                    ones[:],
                    acc_b[:],
                    start=True,
                    stop=True,
                )
                nc.scalar.copy(
                    red_sb[0:1, n_chunks - 1 : n_chunks],
                    red[0:1, n_chunks - 1 : n_chunks],
                )
                nc.sync.dma_start(
                    out_b[:], red_sb[0:1, n_chunks - 1 : n_chunks]
                )

            if loop_n is None:
                body()
            else:
                with tc.For_i(0, loop_n):
                    body()
    return nc


_NC = None


def _get_nc():
    global _NC
    if _NC is None:
        _NC = _hoist_excess_waits(_build())
    return _NC


def kernel(pred: np.ndarray, target: np.ndarray) -> np.ndarray:
    pred = np.ascontiguousarray(np.asarray(pred, dtype=np.float32))
    target = np.ascontiguousarray(np.asarray(target, dtype=np.float32))
    nc = _get_nc()
    in_maps = [
        {
            "pred": pred[i * B_LOC : (i + 1) * B_LOC],
            "target": target[i * B_LOC : (i + 1) * B_LOC],
        }
        for i in range(N_CORES)
    ]
    trace = os.environ.get("DWT_KERNEL_TRACE") == "1"
    core_ids = list(range(N_CORES))
    try:
        res = run_bass_kernel_spmd(nc, in_maps, core_ids=core_ids, trace=trace)
    except ModuleNotFoundError:
        # axon NTFF profile hook unavailable in this environment
        res = run_bass_kernel_spmd(nc, in_maps, core_ids=core_ids, trace=False)
    if trace and res.exec_time_ns is not None:
        print(f"HW exec time: {res.exec_time_ns} ns")
    total = 0.0
    for r in res.results:
        total += float(r["partial_a"].astype(np.float64).sum())
        total += float(r["partial_b"].astype(np.float64).sum())
    return np.float32(total / (2.0 * N_SUB))

